# revision 1
# baseline (speedup 1.0000x reference)
"""Trainium2 Bass kernel for nn_GCNModel (2-layer GCN + sigmoid head).

The network is linear up to the final sigmoid:
    out = sigmoid(M^2 x (W1 W2 Wf) + (M 1)(b1^T W2 Wf) + (b2^T Wf + bf)) * 10
with M = D^-1/2 (A + I) D^-1/2 fixed by the graph. Per core (node-sharded):
    z = x @ w            (w = W1 W2 Wf, PE matvec over the core's x shard)
    u = M z, v = M u     (1-feature SpMVs: per-slot-column indirect-DMA
                          element gathers + fused DVE dot/segmented reduce;
                          AllGather of the 1-feature vector between hops)
    g = M 1              (reduce of the slot coefficients, no gather)
    out = sigmoid(v + c1 g + c2) * 10
Host work is graph preprocessing only: gcn_norm coefficients, a
degree-balanced node permutation, slot layout, and weight folding.
"""

import sys

sys.path.insert(0, "/opt/trn_rl_repo")

import numpy as np

import concourse.bass as bass
import concourse.mybir as mybir
import concourse.tile as tile
from concourse import bacc
from concourse.bass_utils import run_bass_kernel_spmd

P = 128
CORES = 8
N_NODES = 50000
TPC = 49                  # tiles per core
NPC = P * TPC             # padded nodes per core (6272)
NFULL = NPC * CORES       # 50176

F32 = mybir.dt.float32
I32 = mybir.dt.int32

_cache = {}


def _build(SL, segs, KE):
    """segs: list of (t0, t1, L) — tiles [t0,t1) have L slot-columns each."""
    nc = bacc.Bacc("TRN2", target_bir_lowering=False, debug=False,
                   num_devices=CORES)
    xT_p = nc.declare_dram_parameter("xT", [P, NPC], F32, isOutput=False)
    wr_p = nc.declare_dram_parameter("wr", [P, NPC], F32, isOutput=False)
    zr_p = nc.declare_dram_parameter("zr", [P, SL], I32, isOutput=False)
    cf_p = nc.declare_dram_parameter("cf", [P, SL], F32, isOutput=False)
    d2_p = nc.declare_dram_parameter("d2", [P, TPC], F32, isOutput=False)
    c1_p = nc.declare_dram_parameter("c1t", [P, TPC], F32, isOutput=False)
    c2_p = nc.declare_dram_parameter("c2t", [P, TPC], F32, isOutput=False)
    out_p = nc.declare_dram_parameter("out", [P, TPC], F32, isOutput=True)

    warm_d = nc.dram_tensor("warm_d", [P, 1], F32)
    warm_o = nc.dram_tensor("warm_o", [P * CORES, 1], F32, addr_space="Shared")
    zloc_d = nc.dram_tensor("zloc_d", [P, TPC], F32)
    uloc_d = nc.dram_tensor("uloc_d", [P, TPC], F32)
    zsh_d = nc.dram_tensor("zsh_d", [P, TPC], F32)
    ush_d = nc.dram_tensor("ush_d", [P, TPC], F32)
    zfull = nc.dram_tensor("zfull", [NFULL, 1], F32, addr_space="Shared")
    ufull = nc.dram_tensor("ufull", [NFULL, 1], F32, addr_space="Shared")
    rg = [list(range(CORES))]
    Lmax = max(L for (_, _, L) in segs)

    with tile.TileContext(nc) as tc:
        with tc.tile_pool(name="sb", bufs=1) as pool:
            xT_sb = pool.tile([P, NPC], F32)
            zr_sb = pool.tile([P, SL], I32)
            cf_sb = pool.tile([P, SL], F32)
            d2_sb = pool.tile([P, TPC], F32)
            wr_sb = pool.tile([P, NPC], F32)
            c1_sb = pool.tile([P, TPC], F32)
            c2_sb = pool.tile([P, TPC], F32)
            xp_sb = pool.tile([P, NPC], F32)
            z_sb = pool.tile([P, TPC], F32)
            g_sb = pool.tile([P, TPC], F32)
            u_sb = pool.tile([P, TPC], F32)
            u2_sb = pool.tile([P, TPC], F32)
            v_sb = pool.tile([P, TPC], F32)
            v2_sb = pool.tile([P, TPC], F32)
            tmp_sb = pool.tile([P, TPC], F32)
            tmpb_sb = pool.tile([P, TPC], F32)
            warm_sb = pool.tile([P, 1], F32)
            pre_sb = pool.tile([P, TPC], F32)
            sig_sb = pool.tile([P, TPC], F32)
            out_sb = pool.tile([P, TPC], F32)
            scr_sb = pool.tile([P, SL], F32)
            vals1_sb = pool.tile([P, SL], F32)
            vals2_sb = pool.tile([P, SL], F32)

            NCH = 4
            chb = [(k * TPC // NCH) * P for k in range(NCH + 1)]
            with nc.named_scope("load"):
                nc.sync.dma_start(zr_sb[:], zr_p[:, :])
                nc.sync.dma_start(cf_sb[:], cf_p[:, :])
                for k in range(NCH):
                    nc.sync.dma_start(xT_sb[:, chb[k]:chb[k + 1]],
                                      xT_p[:, chb[k]:chb[k + 1]])
                    nc.sync.dma_start(wr_sb[:, chb[k]:chb[k + 1]],
                                      wr_p[:, chb[k]:chb[k + 1]])
                nc.sync.dma_start(d2_sb[:], d2_p[:, :])
                nc.sync.dma_start(c1_sb[:], c1_p[:, :])
                nc.sync.dma_start(c2_sb[:], c2_p[:, :])

            with nc.named_scope("warm"):
                nc.vector.memset(warm_sb[:], 0.0)
                nc.sync.dma_start(warm_d[:, :], warm_sb[:, :])
                nc.gpsimd.collective_compute(
                    "AllGather", mybir.AluOpType.bypass, replica_groups=rg,
                    ins=[warm_d.ap().opt()], outs=[warm_o.ap().opt()],
                )

            with nc.named_scope("z"):
                for k in range(NCH):
                    t0c, t1c = k * TPC // NCH, (k + 1) * TPC // NCH
                    nc.vector.tensor_tensor(
                        out=xp_sb[:, chb[k]:chb[k + 1]],
                        in0=xT_sb[:, chb[k]:chb[k + 1]],
                        in1=wr_sb[:, chb[k]:chb[k + 1]],
                        op=mybir.AluOpType.mult)
                    nc.vector.tensor_reduce(
                        out=z_sb[:, t0c:t1c],
                        in_=xp_sb[:, chb[k]:chb[k + 1]]
                            .rearrange("p (t f) -> p t f", t=t1c - t0c),
                        axis=mybir.AxisListType.X,
                        op=mybir.AluOpType.add,
                    )

            with nc.named_scope("g"):
                coloff = 0
                for (t0, t1, L) in segs:
                    nc.vector.tensor_reduce(
                        out=g_sb[:, t0:t1],
                        in_=cf_sb[:, coloff:coloff + (t1 - t0) * L]
                            .rearrange("p (t w) -> p t w", t=t1 - t0),
                        axis=mybir.AxisListType.X,
                        op=mybir.AluOpType.add,
                    )
                    coloff += (t1 - t0) * L
                nc.vector.tensor_tensor(out=g_sb[:], in0=g_sb[:], in1=d2_sb[:],
                                        op=mybir.AluOpType.add)

            with nc.named_scope("cc1"):
                nc.sync.dma_start(zloc_d[:, :], z_sb[:])
                nc.sync.dma_start(zsh_d[:, :], z_sb[:])
                nc.gpsimd.collective_compute(
                    "AllGather", mybir.AluOpType.bypass, replica_groups=rg,
                    ins=[zsh_d.ap().opt()], outs=[zfull.ap().opt()],
                )

            def spmv(table, loc_table, vals_sb, dst_sb, scope):
                with nc.named_scope(scope):
                    for l in range(KE):
                        nc.gpsimd.indirect_dma_start(
                            out=vals_sb[:, l:l + 1],
                            out_offset=None,
                            in_=loc_table[:, :],
                            in_offset=bass.IndirectOffsetOnAxis(
                                ap=zr_sb[:, l:l + 1], axis=0),
                        )
                    for l in range(KE, SL):
                        nc.gpsimd.indirect_dma_start(
                            out=vals_sb[:, l:l + 1],
                            out_offset=None,
                            in_=table[:, :],
                            in_offset=bass.IndirectOffsetOnAxis(
                                ap=zr_sb[:, l:l + 1], axis=0),
                        )
                    coloff = 0
                    for (t0, t1, L) in segs:
                        w = (t1 - t0) * L
                        nc.vector.tensor_tensor(
                            out=scr_sb[:, coloff:coloff + w],
                            in0=vals_sb[:, coloff:coloff + w],
                            in1=cf_sb[:, coloff:coloff + w],
                            op=mybir.AluOpType.mult)
                        nc.vector.tensor_reduce(
                            out=dst_sb[:, t0:t1],
                            in_=scr_sb[:, coloff:coloff + w]
                                .rearrange("p (t w) -> p t w", t=t1 - t0),
                            axis=mybir.AxisListType.X,
                            op=mybir.AluOpType.add,
                        )
                        coloff += w

            spmv(zfull, zloc_d.ap().rearrange("p t -> (p t) ()"), vals1_sb, u_sb, "spmv1")
            with nc.named_scope("self1"):
                nc.vector.tensor_tensor(out=tmp_sb[:], in0=z_sb[:], in1=d2_sb[:],
                                        op=mybir.AluOpType.mult)
                nc.vector.tensor_tensor(out=u2_sb[:], in0=u_sb[:], in1=tmp_sb[:],
                                        op=mybir.AluOpType.add)

            with nc.named_scope("cc2"):
                nc.sync.dma_start(uloc_d[:, :], u2_sb[:])
                nc.sync.dma_start(ush_d[:, :], u2_sb[:])
                nc.gpsimd.collective_compute(
                    "AllGather", mybir.AluOpType.bypass, replica_groups=rg,
                    ins=[ush_d.ap().opt()], outs=[ufull.ap().opt()],
                )

            spmv(ufull, uloc_d.ap().rearrange("p t -> (p t) ()"), vals2_sb, v_sb, "spmv2")
            with nc.named_scope("fin"):
                nc.vector.tensor_tensor(out=tmpb_sb[:], in0=u2_sb[:], in1=d2_sb[:],
                                        op=mybir.AluOpType.mult)
                nc.vector.tensor_tensor(out=v2_sb[:], in0=v_sb[:], in1=tmpb_sb[:],
                                        op=mybir.AluOpType.add)
                nc.vector.tensor_tensor(out=pre_sb[:], in0=g_sb[:],
                                        in1=c1_sb[:], op=mybir.AluOpType.mult)
                nc.vector.tensor_tensor(out=pre_sb[:], in0=pre_sb[:],
                                        in1=c2_sb[:], op=mybir.AluOpType.add)
                nc.vector.tensor_tensor(out=pre_sb[:], in0=pre_sb[:],
                                        in1=v2_sb[:], op=mybir.AluOpType.add)
                nc.scalar.activation(out=sig_sb[:], in_=pre_sb[:],
                                     func=mybir.ActivationFunctionType.Sigmoid)
                nc.vector.tensor_scalar_mul(out=out_sb[:], in0=sig_sb[:],
                                            scalar1=10.0)
                nc.sync.dma_start(out_p[:, :], out_sb[:])

    nc.compile()
    return nc


def _preprocess(x, edge_index, edge_weight, W1, b1, W2, b2, Wf, bf):
    src = np.asarray(edge_index[0], dtype=np.int64)
    dst = np.asarray(edge_index[1], dtype=np.int64)
    ew = np.asarray(edge_weight, dtype=np.float64)
    N = x.shape[0]

    deg = np.bincount(dst, weights=ew, minlength=N) + 1.0
    dinv = (1.0 / np.sqrt(deg)).astype(np.float32)
    ne = (dinv[src].astype(np.float64) * ew * dinv[dst]).astype(np.float32)

    cnt = np.bincount(dst, minlength=N)
    order = np.argsort(-cnt, kind="stable")      # global rank -> node id
    node_core = np.empty(N, np.int64)
    node_crank = np.empty(N, np.int64)
    node_core[order] = np.arange(N) % CORES
    node_crank[order] = np.arange(N) // CORES    # 0..6249
    node_t = node_crank // P
    node_p = node_crank % P
    node_g = node_core * NPC + 49 * node_p + node_t

    # per-tile max in-degree over all cores (uniform layout across cores)
    Lt = np.ones(TPC, np.int64)
    tp_of_node = node_t
    for t in range(TPC):
        sel = tp_of_node == t
        if sel.any():
            Lt[t] = max(1, cnt[sel].max())

    # segments of consecutive tiles sharing L
    segs = []
    t = 0
    while t < TPC:
        t1 = t
        while t1 < TPC and Lt[t1] == Lt[t]:
            t1 += 1
        segs.append((t, t1, int(Lt[t])))
        t = t1
    SL = int(sum((t1 - t0) * L for (t0, t1, L) in segs))

    tile_off = np.zeros(TPC, np.int64)
    co = 0
    for (t0, t1, L) in segs:
        for t in range(t0, t1):
            tile_off[t] = co
            co += L
    assert co == SL

    KE = 32
    zrow = np.zeros((CORES, P, SL), np.int32)
    coef = np.zeros((CORES, P, SL), np.float32)

    e_core = node_core[dst]
    e_p = node_p[dst]
    e_t = node_t[dst]
    e_gs = node_g[src]
    key = (e_core * TPC * P + e_t * P + e_p).astype(np.int64)
    order_e = np.argsort(key, kind="stable")
    ks = key[order_e]
    first = np.ones(len(ks), bool)
    first[1:] = ks[1:] != ks[:-1]
    starts = np.where(first, np.arange(len(ks)), 0)
    starts = np.maximum.accumulate(starts)
    slot_l = np.arange(len(ks)) - starts
    # within each (core,p,tile) row, order edges local-src-first
    e_local = (node_core[src] == e_core).astype(np.int64)
    order_e2 = np.lexsort((1 - e_local, key))
    ks2 = key[order_e2]
    first2 = np.ones(len(ks2), bool)
    first2[1:] = ks2[1:] != ks2[:-1]
    starts2 = np.maximum.accumulate(np.where(first2, np.arange(len(ks2)), 0))
    slot_l = np.arange(len(ks2)) - starts2
    order_e = order_e2
    cc_ = e_core[order_e]
    pp_ = e_p[order_e]
    tt_ = e_t[order_e]
    lc_ = e_local[order_e]
    col = tile_off[tt_] + slot_l
    gs_ = e_gs[order_e]
    # local slots use local index j (= g - core*NPC) into the local table
    j_ = gs_ - (node_core[src])[order_e] * NPC
    islocal_slot = np.zeros((CORES, P, SL), bool)
    islocal_slot[:] = True          # pad slots count as local (idx 0, coef 0)
    islocal_slot[cc_, pp_, col] = lc_.astype(bool)
    # KE = longest prefix of columns fully local across all cores/partitions
    colOK = islocal_slot.all(axis=(0, 1))
    KE = 0
    while KE < SL and colOK[KE]:
        KE += 1
    KE = min(KE, SL)
    zrow[cc_, pp_, col] = np.where(lc_.astype(bool) & (col < KE), j_, gs_).astype(np.int32)
    coef[cc_, pp_, col] = ne[order_e]

    d2 = np.zeros((CORES, P, TPC), np.float32)
    d2[node_core, node_p, node_t] = (dinv * dinv).astype(np.float32)
    xT = np.zeros((CORES, P, TPC, P), np.float32)
    xT[node_core, node_p, node_t, :] = np.asarray(x, np.float32)
    xT = xT.reshape(CORES, P, NPC)

    W1 = np.asarray(W1, np.float64)
    W2 = np.asarray(W2, np.float64)
    Wf = np.asarray(Wf, np.float64)
    w = (W1 @ W2 @ Wf).astype(np.float32)            # [128,1]
    c1 = float((np.asarray(b1, np.float64) @ W2 @ Wf).reshape(()))
    c2 = float((np.asarray(b2, np.float64) @ Wf).reshape(()) + float(np.asarray(bf, np.float64).reshape(())))

    return dict(SL=SL, segs=segs, KE=KE, zrow=zrow, coef=coef, d2=d2,
                xT=xT, w=w, c1=c1, c2=c2, node_core=node_core,
                node_p=node_p, node_t=node_t)


def kernel(x, edge_index, edge_weight, W1, b1, W2, b2, Wf, bf):
    pp = _preprocess(x, edge_index, edge_weight, W1, b1, W2, b2, Wf, bf)
    key = (pp["SL"], tuple(pp["segs"]), pp["KE"])
    if key not in _cache:
        _cache[key] = _build(pp["SL"], pp["segs"], pp["KE"])
    nc = _cache[key]

    c1t = np.full((P, TPC), pp["c1"], np.float32)
    c2t = np.full((P, TPC), pp["c2"], np.float32)
    wr = np.tile(pp["w"].reshape(1, P), (P, TPC)).reshape(P, NPC)

    in_maps = []
    for c in range(CORES):
        in_maps.append({
            "xT": np.ascontiguousarray(pp["xT"][c]),
            "zr": np.ascontiguousarray(pp["zrow"][c]),
            "cf": np.ascontiguousarray(pp["coef"][c]),
            "d2": np.ascontiguousarray(pp["d2"][c]),
            "wr": wr,
            "c1t": c1t,
            "c2t": c2t,
        })
    res = run_bass_kernel_spmd(nc, in_maps, core_ids=list(range(CORES)))
    outs = np.stack([res.results[c]["out"] for c in range(CORES)])
    out = np.empty((N_NODES, 1), np.float32)
    out[:, 0] = outs[pp["node_core"], pp["node_p"], pp["node_t"]]
    return out



# revision 17
# speedup vs baseline: 1.0340x; 1.0340x over previous
"""Trainium2 Bass kernel for nn_GCNModel (2-layer GCN + sigmoid head).

The network is linear up to the final sigmoid:
    out = sigmoid(M^2 x (W1 W2 Wf) + (M 1)(b1^T W2 Wf) + (b2^T Wf + bf)) * 10
with M = D^-1/2 (A + I) D^-1/2 fixed by the graph. Per core (node-sharded):
    z = x @ w            (w = W1 W2 Wf, PE matvec over the core's x shard)
    u = M z, v = M u     (1-feature SpMVs: per-slot-column indirect-DMA
                          element gathers + fused DVE dot/segmented reduce;
                          AllGather of the 1-feature vector between hops)
    g = M 1              (reduce of the slot coefficients, no gather)
    out = sigmoid(v + c1 g + c2) * 10
Host work is graph preprocessing only: gcn_norm coefficients, a
degree-balanced node permutation, slot layout, and weight folding.
"""

import sys

sys.path.insert(0, "/opt/trn_rl_repo")

import numpy as np

import concourse.bass as bass
import concourse.mybir as mybir
import concourse.tile as tile
from concourse import bacc
from concourse.bass_utils import run_bass_kernel_spmd

P = 128
CORES = 8
N_NODES = 50000
TPC = 49                  # tiles per core
NPC = P * TPC             # padded nodes per core (6272)
NFULL = NPC * CORES       # 50176

F32 = mybir.dt.float32
I32 = mybir.dt.int32

_cache = {}


def _build(SL, segs, KE):
    """segs: list of (t0, t1, L) — tiles [t0,t1) have L slot-columns each."""
    nc = bacc.Bacc("TRN2", target_bir_lowering=False, debug=False,
                   num_devices=CORES)
    xT_p = nc.declare_dram_parameter("xT", [P, NPC], F32, isOutput=False)
    wr_p = nc.declare_dram_parameter("wr", [P, NPC], F32, isOutput=False)
    zr_p = nc.declare_dram_parameter("zr", [P, SL], I32, isOutput=False)
    cf_p = nc.declare_dram_parameter("cf", [P, SL], F32, isOutput=False)
    d2_p = nc.declare_dram_parameter("d2", [P, TPC], F32, isOutput=False)
    c1_p = nc.declare_dram_parameter("c1t", [P, TPC], F32, isOutput=False)
    c2_p = nc.declare_dram_parameter("c2t", [P, TPC], F32, isOutput=False)
    out_p = nc.declare_dram_parameter("out", [P, TPC], F32, isOutput=True)

    warm_d = nc.dram_tensor("warm_d", [P, 1], F32)
    warm_o = nc.dram_tensor("warm_o", [P * CORES, 1], F32, addr_space="Shared")
    zloc_d = nc.dram_tensor("zloc_d", [P, TPC], F32)
    uloc_d = nc.dram_tensor("uloc_d", [P, TPC], F32)
    zsh_d = nc.dram_tensor("zsh_d", [P, TPC], F32)
    ush_d = nc.dram_tensor("ush_d", [P, TPC], F32)
    zfull = nc.dram_tensor("zfull", [NFULL, 1], F32, addr_space="Shared")
    ufull = nc.dram_tensor("ufull", [NFULL, 1], F32, addr_space="Shared")
    rg = [list(range(CORES))]
    Lmax = max(L for (_, _, L) in segs)

    with tile.TileContext(nc) as tc:
        with tc.tile_pool(name="sb", bufs=1) as pool:
            xT_sb = pool.tile([P, NPC], F32)
            zr_sb = pool.tile([P, SL], I32)
            cf_sb = pool.tile([P, SL], F32)
            d2_sb = pool.tile([P, TPC], F32)
            wr_sb = pool.tile([P, NPC], F32)
            c1_sb = pool.tile([P, TPC], F32)
            c2_sb = pool.tile([P, TPC], F32)
            xp_sb = pool.tile([P, NPC], F32)
            z_sb = pool.tile([P, TPC], F32)
            g_sb = pool.tile([P, TPC], F32)
            u_sb = pool.tile([P, TPC], F32)
            u2_sb = pool.tile([P, TPC], F32)
            v_sb = pool.tile([P, TPC], F32)
            v2_sb = pool.tile([P, TPC], F32)
            tmp_sb = pool.tile([P, TPC], F32)
            tmpb_sb = pool.tile([P, TPC], F32)
            warm_sb = pool.tile([P, 1], F32)
            pre_sb = pool.tile([P, TPC], F32)
            sig_sb = pool.tile([P, TPC], F32)
            out_sb = pool.tile([P, TPC], F32)
            scr_sb = pool.tile([P, SL], F32)
            vals1_sb = pool.tile([P, SL], F32)
            vals2_sb = pool.tile([P, SL], F32)

            NCH = 4
            chb = [(k * TPC // NCH) * P for k in range(NCH + 1)]
            with nc.named_scope("load"):
                nc.sync.dma_start(zr_sb[:], zr_p[:, :])
                nc.sync.dma_start(cf_sb[:], cf_p[:, :])
                for k in range(NCH):
                    nc.sync.dma_start(xT_sb[:, chb[k]:chb[k + 1]],
                                      xT_p[:, chb[k]:chb[k + 1]])
                    nc.sync.dma_start(wr_sb[:, chb[k]:chb[k + 1]],
                                      wr_p[:, chb[k]:chb[k + 1]])
                nc.sync.dma_start(d2_sb[:], d2_p[:, :])
                nc.sync.dma_start(c1_sb[:], c1_p[:, :])
                nc.sync.dma_start(c2_sb[:], c2_p[:, :])

            with nc.named_scope("warm"):
                nc.vector.memset(warm_sb[:], 0.0)
                nc.sync.dma_start(warm_d[:, :], warm_sb[:, :])
                nc.gpsimd.collective_compute(
                    "AllGather", mybir.AluOpType.bypass, replica_groups=rg,
                    ins=[warm_d.ap().opt()], outs=[warm_o.ap().opt()],
                )

            with nc.named_scope("z"):
                for k in range(NCH):
                    t0c, t1c = k * TPC // NCH, (k + 1) * TPC // NCH
                    nc.vector.tensor_tensor(
                        out=xp_sb[:, chb[k]:chb[k + 1]],
                        in0=xT_sb[:, chb[k]:chb[k + 1]],
                        in1=wr_sb[:, chb[k]:chb[k + 1]],
                        op=mybir.AluOpType.mult)
                    nc.vector.tensor_reduce(
                        out=z_sb[:, t0c:t1c],
                        in_=xp_sb[:, chb[k]:chb[k + 1]]
                            .rearrange("p (t f) -> p t f", t=t1c - t0c),
                        axis=mybir.AxisListType.X,
                        op=mybir.AluOpType.add,
                    )

            with nc.named_scope("g"):
                coloff = 0
                for (t0, t1, L) in segs:
                    nc.vector.tensor_reduce(
                        out=g_sb[:, t0:t1],
                        in_=cf_sb[:, coloff:coloff + (t1 - t0) * L]
                            .rearrange("p (t w) -> p t w", t=t1 - t0),
                        axis=mybir.AxisListType.X,
                        op=mybir.AluOpType.add,
                    )
                    coloff += (t1 - t0) * L
                nc.vector.tensor_tensor(out=g_sb[:], in0=g_sb[:], in1=d2_sb[:],
                                        op=mybir.AluOpType.add)

            with nc.named_scope("cc1"):
                nc.sync.dma_start(zloc_d[:, :], z_sb[:])
                nc.sync.dma_start(zsh_d[:, :], z_sb[:])
                nc.gpsimd.collective_compute(
                    "AllGather", mybir.AluOpType.bypass, replica_groups=rg,
                    ins=[zsh_d.ap().opt()], outs=[zfull.ap().opt()],
                )

            def spmv(table, loc_table, vals_sb, dst_sb, scope):
                with nc.named_scope(scope):
                    for l in range(KE):
                        nc.gpsimd.indirect_dma_start(
                            out=vals_sb[:, l:l + 1],
                            out_offset=None,
                            in_=loc_table[:, :],
                            in_offset=bass.IndirectOffsetOnAxis(
                                ap=zr_sb[:, l:l + 1], axis=0),
                        )
                    for l in range(KE, SL):
                        nc.gpsimd.indirect_dma_start(
                            out=vals_sb[:, l:l + 1],
                            out_offset=None,
                            in_=table[:, :],
                            in_offset=bass.IndirectOffsetOnAxis(
                                ap=zr_sb[:, l:l + 1], axis=0),
                        )
                    coloff = 0
                    for (t0, t1, L) in segs:
                        w = (t1 - t0) * L
                        nc.vector.tensor_tensor(
                            out=scr_sb[:, coloff:coloff + w],
                            in0=vals_sb[:, coloff:coloff + w],
                            in1=cf_sb[:, coloff:coloff + w],
                            op=mybir.AluOpType.mult)
                        nc.vector.tensor_reduce(
                            out=dst_sb[:, t0:t1],
                            in_=scr_sb[:, coloff:coloff + w]
                                .rearrange("p (t w) -> p t w", t=t1 - t0),
                            axis=mybir.AxisListType.X,
                            op=mybir.AluOpType.add,
                        )
                        coloff += w

            spmv(zfull, zloc_d.ap().rearrange("p t -> (p t) ()"), vals1_sb, u_sb, "spmv1")
            with nc.named_scope("self1"):
                nc.vector.tensor_tensor(out=tmp_sb[:], in0=z_sb[:], in1=d2_sb[:],
                                        op=mybir.AluOpType.mult)
                nc.vector.tensor_tensor(out=u2_sb[:], in0=u_sb[:], in1=tmp_sb[:],
                                        op=mybir.AluOpType.add)

            with nc.named_scope("cc2"):
                nc.sync.dma_start(uloc_d[:, :], u2_sb[:])
                nc.sync.dma_start(ush_d[:, :], u2_sb[:])
                nc.gpsimd.collective_compute(
                    "AllGather", mybir.AluOpType.bypass, replica_groups=rg,
                    ins=[ush_d.ap().opt()], outs=[ufull.ap().opt()],
                )

            spmv(ufull, uloc_d.ap().rearrange("p t -> (p t) ()"), vals2_sb, v_sb, "spmv2")
            with nc.named_scope("fin"):
                nc.vector.tensor_tensor(out=tmpb_sb[:], in0=u2_sb[:], in1=d2_sb[:],
                                        op=mybir.AluOpType.mult)
                nc.vector.tensor_tensor(out=v2_sb[:], in0=v_sb[:], in1=tmpb_sb[:],
                                        op=mybir.AluOpType.add)
                nc.vector.tensor_tensor(out=pre_sb[:], in0=g_sb[:],
                                        in1=c1_sb[:], op=mybir.AluOpType.mult)
                nc.vector.tensor_tensor(out=pre_sb[:], in0=pre_sb[:],
                                        in1=c2_sb[:], op=mybir.AluOpType.add)
                nc.vector.tensor_tensor(out=pre_sb[:], in0=pre_sb[:],
                                        in1=v2_sb[:], op=mybir.AluOpType.add)
                nc.scalar.activation(out=sig_sb[:], in_=pre_sb[:],
                                     func=mybir.ActivationFunctionType.Sigmoid)
                nc.vector.tensor_scalar_mul(out=out_sb[:], in0=sig_sb[:],
                                            scalar1=10.0)
                nc.sync.dma_start(out_p[:, :], out_sb[:])

    nc.compile()
    return nc


def _preprocess(x, edge_index, edge_weight, W1, b1, W2, b2, Wf, bf):
    src = np.asarray(edge_index[0], dtype=np.int64)
    dst = np.asarray(edge_index[1], dtype=np.int64)
    ew = np.asarray(edge_weight, dtype=np.float64)
    N = x.shape[0]

    deg = np.bincount(dst, weights=ew, minlength=N) + 1.0
    dinv = (1.0 / np.sqrt(deg)).astype(np.float32)
    ne = (dinv[src].astype(np.float64) * ew * dinv[dst]).astype(np.float32)

    cnt = np.bincount(dst, minlength=N)
    order = np.argsort(-cnt, kind="stable")      # global rank -> node id
    node_core = np.empty(N, np.int64)
    node_crank = np.empty(N, np.int64)
    node_core[order] = np.arange(N) % CORES
    node_crank[order] = np.arange(N) // CORES    # 0..6249
    node_t = node_crank // P
    node_p = node_crank % P
    node_g = node_core * NPC + 49 * node_p + node_t

    # per-tile max in-degree over all cores (uniform layout across cores)
    Lt = np.ones(TPC, np.int64)
    tp_of_node = node_t
    for t in range(TPC):
        sel = tp_of_node == t
        if sel.any():
            Lt[t] = max(1, cnt[sel].max())

    # segments of consecutive tiles sharing L
    segs = []
    t = 0
    while t < TPC:
        t1 = t
        while t1 < TPC and Lt[t1] == Lt[t]:
            t1 += 1
        segs.append((t, t1, int(Lt[t])))
        t = t1
    SL = int(sum((t1 - t0) * L for (t0, t1, L) in segs))

    tile_off = np.zeros(TPC, np.int64)
    co = 0
    for (t0, t1, L) in segs:
        for t in range(t0, t1):
            tile_off[t] = co
            co += L
    assert co == SL

    KE = 32
    zrow = np.zeros((CORES, P, SL), np.int32)
    coef = np.zeros((CORES, P, SL), np.float32)

    e_core = node_core[dst]
    e_p = node_p[dst]
    e_t = node_t[dst]
    e_gs = node_g[src]
    key = (e_core * TPC * P + e_t * P + e_p).astype(np.int64)
    order_e = np.argsort(key, kind="stable")
    ks = key[order_e]
    first = np.ones(len(ks), bool)
    first[1:] = ks[1:] != ks[:-1]
    starts = np.where(first, np.arange(len(ks)), 0)
    starts = np.maximum.accumulate(starts)
    slot_l = np.arange(len(ks)) - starts
    # within each (core,p,tile) row, order edges local-src-first
    e_local = (node_core[src] == e_core).astype(np.int64)
    order_e2 = np.lexsort((1 - e_local, key))
    ks2 = key[order_e2]
    first2 = np.ones(len(ks2), bool)
    first2[1:] = ks2[1:] != ks2[:-1]
    starts2 = np.maximum.accumulate(np.where(first2, np.arange(len(ks2)), 0))
    slot_l = np.arange(len(ks2)) - starts2
    order_e = order_e2
    cc_ = e_core[order_e]
    pp_ = e_p[order_e]
    tt_ = e_t[order_e]
    lc_ = e_local[order_e]
    col = tile_off[tt_] + slot_l
    gs_ = e_gs[order_e]
    # local slots use local index j (= g - core*NPC) into the local table
    j_ = gs_ - (node_core[src])[order_e] * NPC
    islocal_slot = np.zeros((CORES, P, SL), bool)
    islocal_slot[:] = True          # pad slots count as local (idx 0, coef 0)
    islocal_slot[cc_, pp_, col] = lc_.astype(bool)
    # KE = longest prefix of columns fully local across all cores/partitions
    colOK = islocal_slot.all(axis=(0, 1))
    KE = 0
    while KE < SL and colOK[KE]:
        KE += 1
    KE = min(KE, SL)
    zrow[cc_, pp_, col] = np.where(lc_.astype(bool) & (col < KE), j_, gs_).astype(np.int32)
    coef[cc_, pp_, col] = ne[order_e]

    d2 = np.zeros((CORES, P, TPC), np.float32)
    d2[node_core, node_p, node_t] = (dinv * dinv).astype(np.float32)
    xT = np.zeros((CORES, P, TPC, P), np.float32)
    xT[node_core, node_p, node_t, :] = np.asarray(x, np.float32)
    xT = xT.reshape(CORES, P, NPC)

    W1 = np.asarray(W1, np.float64)
    W2 = np.asarray(W2, np.float64)
    Wf = np.asarray(Wf, np.float64)
    w = (W1 @ W2 @ Wf).astype(np.float32)            # [128,1]
    c1 = float((np.asarray(b1, np.float64) @ W2 @ Wf).reshape(()))
    c2 = float((np.asarray(b2, np.float64) @ Wf).reshape(()) + float(np.asarray(bf, np.float64).reshape(())))

    return dict(SL=SL, segs=segs, KE=KE, zrow=zrow, coef=coef, d2=d2,
                xT=xT, w=w, c1=c1, c2=c2, node_core=node_core,
                node_p=node_p, node_t=node_t)


def kernel(x, edge_index, edge_weight, W1, b1, W2, b2, Wf, bf):
    pp = _preprocess(x, edge_index, edge_weight, W1, b1, W2, b2, Wf, bf)
    key = (pp["SL"], tuple(pp["segs"]), pp["KE"])
    if key not in _cache:
        _cache[key] = _build(pp["SL"], pp["segs"], pp["KE"])
    nc = _cache[key]

    c1t = np.full((P, TPC), pp["c1"], np.float32)
    c2t = np.full((P, TPC), pp["c2"], np.float32)
    wr = np.tile(pp["w"].reshape(1, P), (P, TPC)).reshape(P, NPC)

    in_maps = []
    for c in range(CORES):
        in_maps.append({
            "xT": np.ascontiguousarray(pp["xT"][c]),
            "zr": np.ascontiguousarray(pp["zrow"][c]),
            "cf": np.ascontiguousarray(pp["coef"][c]),
            "d2": np.ascontiguousarray(pp["d2"][c]),
            "wr": wr,
            "c1t": c1t,
            "c2t": c2t,
        })
    res = run_bass_kernel_spmd(nc, in_maps, core_ids=list(range(CORES)))
    outs = np.stack([res.results[c]["out"] for c in range(CORES)])
    out = np.empty((N_NODES, 1), np.float32)
    out[:, 0] = outs[pp["node_core"], pp["node_p"], pp["node_t"]]
    return out

